# revision 32
# baseline (speedup 1.0000x reference)
"""Char-LSTM kernel for Trainium2 (8 NeuronCores, data parallel).

Strategy
--------
Host side:
  * Global descending-length fill: sort words by length desc, cut into 16
    chunks of 4096 (512/core); chunk run length = its longest word. This
    meets the pair-capacity lower bound on total group-steps (75 here).
    Mixed-length blocks capture h at each word's final step.
  * Per (block, t>0) ship an x-slab [64, 512] bf16 = [ones | emb[ch].T |
    zeros] so the whole gate pre-activation is ONE K=128 matmul per bank:
    rhs = [h | ones | x | zeros], lhsT = [W_hh.T | b | W_ih.T | 0]. (The
    zero padding keeps the moving operand partition-base 32-aligned, which
    the ISA requires.)
  * t = 0 needs NO activations at all: c0 = (sig(2Gg)-0.5)*sig(Gi) and
    h0 = sig(Go)*tanh(2*c0) are pure functions of the first char, shipped
    as 64-wide tables fetched exactly by a one-hot matmul.

Device side per group-step (two 512-word blocks A/B sharing 128 partitions):
  * 8 matmuls (4 gate banks x 2 halves, M=64, K=128, N=512) into one
    [128, 2048] PSUM tile laid out as banks [i | f | o | g].
  * ONE sigmoid over all 4 banks [128, 2048] -> bf16 SBUF. The g-bank
    weights are pre-scaled by 2 so tanh(g) = 2*sigmoid(2g) - 1; the cell
    state is kept halved (c' = c/2) which makes the update exact:
        t1 = (sg - 0.5) * si            (= i*g / 2, fused DVE op)
        c' = sf * c' + t1               (all bf16 -> DVE 2x rate)
        tc = tanh(2 * c')               (ONE act instr, scale=2)
        h  = so * tc                    (bf16, written straight into the
                                         next step's matmul rhs tile)
  * h lives in rows 0:64 of the A-slab and rows 64:128 of the B-slab, so
    no partition shifts are ever needed.
  Groups run 9-deep interleaved (fixed active set, round-robin) to hide the
  recurrence-chain latency; all cell-update ops stay on the DVE because a
  cross-engine hop (Pool) costs two semaphore round-trips on the critical
  path (~26us measured). HW-measured ablations showed the kernel is bound
  by chain latency + the 2-deep PSUM recycle loop, not by any one engine's
  throughput (Act busy ~170us of the ~195us wall).
"""

import os
import sys

for _p in ("/opt/trn_rl_repo", "/root/.axon_site/_ro/trn_rl_repo"):
    if os.path.isdir(_p) and _p not in sys.path:
        sys.path.insert(0, _p)

import numpy as np
import ml_dtypes

BF16 = ml_dtypes.bfloat16

H = 64          # hidden size
E = 32          # char embedding size
V = 100         # vocab
MAXL = 16       # max word length
BLK = 512       # words per block (one half of a group)
NCORES = 8
GATE4 = 4 * H   # 256
KDIM = H + 1 + E  # 97: [h | ones | x]

# torch gate order in the weights is [i, f, g, o]; we stage banks as
# [i, f, o, g] so one sigmoid covers the whole [128, 2048] span (g uses the
# 2*sigmoid(2x)-1 tanh identity).
_GATE_PERM = (np.concatenate([
    np.arange(0, 64),        # i
    np.arange(128, 192),     # g
    np.arange(64, 128),      # f
    np.arange(192, 256),     # o
]) if os.environ.get("LSTM_SIGSPLIT", "0") == "1" else np.concatenate([
    np.arange(0, 64),        # i
    np.arange(64, 128),      # f
    np.arange(192, 256),     # o
    np.arange(128, 192),     # g
]))
_G_BANK = 1 if os.environ.get("LSTM_SIGSPLIT", "0") == "1" else 3

INTERLEAVE = int(os.environ.get("LSTM_INTERLEAVE", "9"))
SKIPB = os.environ.get("LSTM_SKIPB", "1") == "1"
T0TAB = os.environ.get("LSTM_T0TAB", "1") == "1"
PAIRTANH = os.environ.get("LSTM_PAIRTANH", "0") == "1"
SIGSPLIT = os.environ.get("LSTM_SIGSPLIT", "0") == "1"
T0ACT = os.environ.get("LSTM_T0ACT", "0") == "1"
_PROGRAM_CACHE = {}


# --------------------------------------------------------------------------
# Host-side planning
# --------------------------------------------------------------------------

def _plan(lengths):
    """Assign words to (core, block, column) slots.

    Returns dict with:
      blocks: list (shared across cores) of dicts {L, is_ov, ov_idx}
      groups: list of dicts {a, b, steps} (block indices)
      sched:  emission order list of (group_idx, t)
      assign: per core: list of np arrays [BLK] of word ids (-1 = dummy),
              aligned with blocks
    """
    n = lengths.shape[0]
    lengths = lengths.astype(np.int64)
    # Global descending-length fill: cut the sorted word list into chunks of
    # NCORES*BLK; chunk i becomes block i with run length = its longest word.
    # This provably minimizes the paired sum-of-maxes (the pair-capacity
    # bound is met with equality) and leaves no dummy slots except in the
    # final chunk. Mixed-length blocks run per-step capture.
    order = np.argsort(-lengths, kind="stable")
    nb = -(-n // (NCORES * BLK))
    if nb % 2 == 1:
        nb += 1
    blocks = []
    assign = [[] for _ in range(NCORES)]
    ov_count = 0
    for i in range(nb):
        chunk = order[i * NCORES * BLK:(i + 1) * NCORES * BLK]
        arr = np.full(NCORES * BLK, -1, dtype=np.int64)
        arr[: chunk.shape[0]] = chunk
        arr = arr.reshape(NCORES, BLK)
        if chunk.shape[0]:
            run = int(lengths[chunk].max())
            is_ov = bool(lengths[chunk].min() < run) or chunk.shape[0] < NCORES * BLK
        else:
            run, is_ov = 1, False
        blocks.append({"L": run, "is_ov": is_ov,
                       "ov_idx": (ov_count if is_ov else -1)})
        if is_ov:
            ov_count += 1
        for c in range(NCORES):
            assign[c].append(arr[c])

    groups = []
    for i in range(0, nb, 2):
        groups.append({"a": i, "b": i + 1,
                       "steps": max(blocks[i]["L"], blocks[i + 1]["L"])})

    # Fixed-active-set interleave over PARTNER PAIRS of groups. Partners run
    # in lockstep so one tanh instruction can cover both groups' cell states
    # ([128, 1024] vs 2x[128, 512] -- halves the per-instr activation
    # overhead). sched entries are (g, t, mode): "defer" = first partner of
    # a both-alive step (tanh/h/out postponed), "shared" = second partner
    # (runs the pair-wide tanh + both tails), "solo" = no live partner.
    steps = [g["steps"] for g in groups]
    sched = []
    if PAIRTANH:
        pairs = [(i, i + 1 if i + 1 < len(groups) else None)
                 for i in range(0, len(groups), 2)]
        slots = max(1, INTERLEAVE // 2)
        active = pairs[:slots]
        queue = pairs[slots:]
        pt = {p: 0 for p in pairs}
        while active:
            for pair in list(active):
                gA, gB = pair
                t = pt[pair]
                aliveA = t < steps[gA]
                aliveB = gB is not None and t < steps[gB]
                if aliveA and aliveB and t > 0:
                    sched.append((gA, t, "defer"))
                    sched.append((gB, t, "shared"))
                else:
                    if aliveA:
                        sched.append((gA, t, "solo"))
                    if aliveB:
                        sched.append((gB, t, "solo"))
                pt[pair] += 1
                done = pt[pair] >= max(steps[gA],
                                       steps[gB] if gB is not None else 0)
                if done:
                    i = active.index(pair)
                    if queue:
                        active[i] = queue.pop(0)
                    else:
                        active.pop(i)
    else:
        remaining = list(steps)
        next_t = [0] * len(groups)
        queue = sorted(range(len(groups)), key=lambda g: -remaining[g])
        active = queue[:INTERLEAVE]
        queue = queue[INTERLEAVE:]
        while active:
            for g in list(active):
                sched.append((g, next_t[g], "solo"))
                next_t[g] += 1
                remaining[g] -= 1
                if remaining[g] == 0:
                    i = active.index(g)
                    if queue:
                        active[i] = queue.pop(0)
                    else:
                        active.pop(i)

    # capture steps: for each capture block, the union (over cores) of
    # final steps of its words with length < MAXL, plus MAXL-1 (so length-16
    # words folded into a capture block are also covered).
    for bi, blk in enumerate(blocks):
        if not blk["is_ov"]:
            continue
        steps = set()
        for c in range(NCORES):
            w = assign[c][bi]
            w = w[w >= 0]
            steps.update((lengths[w] - 1).tolist())
        blk["cap_steps"] = tuple(sorted(steps))

    return {"blocks": blocks, "groups": groups, "sched": sched,
            "assign": assign, "n_ov": ov_count}


def _build_xslabs(plan, chars, emb16):
    """Per-core x-slab tensors [n_blocks*MAXL, 64, BLK] bf16, indexed by
    block_idx*MAXL + t. The 64-row slab is DMA'd whole so the matmul K range
    is a full, partition-aligned 128 rows (zero rows x zero weights = 0).

    A-half blocks (even index), DMA'd to partitions 64:128:
      row 0 = ones, rows 1:33 = emb[ch].T, rows 33:64 = zeros.
    B-half blocks (odd index), DMA'd to partitions 0:64:
      rows 0:32 = emb[ch].T, row 32 = ones, rows 33:64 = zeros.
    """
    blocks = plan["blocks"]
    nb = len(blocks)
    out = []
    for c in range(NCORES):
        xs = np.zeros((nb * MAXL, 64, BLK), dtype=BF16)
        for bi, blk in enumerate(blocks):
            words = plan["assign"][c][bi]
            w = np.where(words < 0, 0, words)
            L = blk["L"]
            ch = chars[w, :L]                       # [BLK, L]
            xt = emb16[ch]                          # [BLK, L, E]
            xt = np.ascontiguousarray(np.transpose(xt, (1, 2, 0)))  # [L, E, BLK]
            base = bi * MAXL
            if bi % 2 == 0:
                xs[base:base + L, 1:33, :] = xt
                xs[base:base + L, 0, :] = 1.0
            else:
                xs[base:base + L, 0:32, :] = xt
                xs[base:base + L, 32, :] = 1.0
        out.append(xs)
    return out


def _build_oh0(plan, chars):
    """Per-core one-hot slabs [n_blocks, 128, BLK] bf16 of each word's FIRST
    char (t=0 table-lookup matmul). Dummy columns stay all-zero."""
    blocks = plan["blocks"]
    nb = len(blocks)
    out = []
    for c in range(NCORES):
        oh = np.zeros((nb, 128, BLK), dtype=BF16)
        for bi in range(nb):
            words = plan["assign"][c][bi]
            valid = words >= 0
            cols = np.nonzero(valid)[0]
            if cols.shape[0]:
                oh[bi, chars[words[valid], 0], cols] = 1.0
        out.append(oh)
    return out


# --------------------------------------------------------------------------
# Device program
# --------------------------------------------------------------------------

def _build_program(plan_sig, blocks, groups, sched, n_ov, variant="full",
                   reps=1):
    import concourse.bass as bass
    import concourse.tile as tile
    from concourse import bacc, mybir
    from contextlib import nullcontext

    do_mm = variant not in ("nomm",)
    do_act = variant not in ("noact",)
    do_dma = variant not in ("nodma",)

    t2dve = variant == "t2dve" or os.environ.get("LSTM_T2DVE", "1") == "1"

    f32 = mybir.dt.float32
    bf16 = mybir.dt.bfloat16
    Sigmoid = mybir.ActivationFunctionType.Sigmoid
    Tanh = mybir.ActivationFunctionType.Tanh
    ADD = mybir.AluOpType.add
    MULT = mybir.AluOpType.mult

    n_blocks = len(blocks)
    n_slabs = n_blocks * MAXL

    nc = bacc.Bacc("TRN2", target_bir_lowering=False, debug=False,
                   num_devices=NCORES)
    xsl_d = nc.dram_tensor("xsl", [n_slabs, 64, BLK], bf16,
                           kind="ExternalInput")
    oh0_d = nc.dram_tensor("oh0", [n_blocks, 128, BLK], bf16,
                           kind="ExternalInput")
    ga_d = nc.dram_tensor("ga", [128, GATE4], bf16, kind="ExternalInput")
    gb_d = nc.dram_tensor("gb", [128, GATE4], bf16, kind="ExternalInput")
    t0_d = nc.dram_tensor("t0", [128, 128], bf16, kind="ExternalInput")
    out_d = nc.dram_tensor("out", [n_blocks, H, BLK], bf16,
                           kind="ExternalOutput")
    ov_d = nc.dram_tensor("ov", [max(1, n_ov) * MAXL, H, BLK], bf16,
                          kind="ExternalOutput")

    with tile.TileContext(nc) as tc:
        with (
            tc.tile_pool(name="consts", bufs=1) as consts,
            tc.tile_pool(name="slabs", bufs=4 * INTERLEAVE + 2) as slabs,
            tc.tile_pool(name="psum", bufs=(4 if SIGSPLIT else 2),
                         space="PSUM") as psump,
            tc.tile_pool(name="sig", bufs=INTERLEAVE + 2) as sigp,
            tc.tile_pool(name="t1_", bufs=4) as t1p,
            tc.tile_pool(name="t2_", bufs=4) as t2p,
            tc.tile_pool(name="tc_", bufs=4) as tcp,
            tc.tile_pool(name="state",
                         bufs=(max(4, INTERLEAVE // 2 + 2) if PAIRTANH
                               else INTERLEAVE + 2)) as statep,
        ):
            ga = consts.tile([128, GATE4], bf16, tag="ga")
            gb = consts.tile([128, GATE4], bf16, tag="gb")
            t0t = consts.tile([128, 128], bf16, tag="t0t")
            nc.sync.dma_start(out=ga[:], in_=ga_d[:])
            nc.sync.dma_start(out=gb[:], in_=gb_d[:])
            nc.sync.dma_start(out=t0t[:], in_=t0_d[:])

            loop_cm = tc.For_i(0, reps, 1) if reps > 1 else nullcontext()
            with loop_cm:
                gstate = {}
                pairc = {}   # pair -> shared c tile [128, 2*BLK]
                pend = {}    # pair -> stashed "defer" context

                def tail(ctx, tch, off):
                    """h = sig_o * tanh(c) into the next rhs tiles + outputs."""
                    sig, na, nb_ = ctx["sig"], ctx["na"], ctx["nb_"]
                    ca, cb = ctx["a"], ctx["b"]
                    cgrp, ct = ctx["grp"], ctx["t"]
                    osl = ctx["osl"]
                    nc.vector.tensor_mul(na[0:64, :],
                                         sig[0:64, osl],
                                         tch[0:64, off:off + BLK])
                    if ctx["b_act"]:
                        nc.vector.tensor_mul(nb_[64:128, :],
                                             sig[64:128, osl],
                                             tch[64:128, off:off + BLK])
                    if do_dma:
                        if ct == ctx["La"] - 1:
                            nc.sync.dma_start(out=out_d[cgrp["a"]],
                                              in_=na[0:64, :])
                        if ctx["b_act"] and ct == ctx["Lb"] - 1:
                            nc.sync.dma_start(out=out_d[cgrp["b"]],
                                              in_=nb_[64:128, :])
                        if ca["is_ov"] and ct in ca.get("cap_steps", ()):
                            nc.sync.dma_start(
                                out=ov_d[ca["ov_idx"] * MAXL + ct],
                                in_=na[0:64, :])
                        if ctx["b_act"] and cb["is_ov"] and \
                                ct in cb.get("cap_steps", ()):
                            nc.sync.dma_start(
                                out=ov_d[cb["ov_idx"] * MAXL + ct],
                                in_=nb_[64:128, :])

                for (g, t, mode) in sched:
                    grp = groups[g]
                    a, b = blocks[grp["a"]], blocks[grp["b"]]
                    La, Lb = a["L"], b["L"]
                    first = (t == 0)
                    b_act = (t < Lb) or not SKIPB
                    sl = slice(0, 128 if b_act else 64)
                    pk = g // 2 if PAIRTANH else g
                    coff = (g % 2) * BLK if PAIRTANH else 0

                    if first:
                        # t == 0: c0 and h0 are pure functions of the first
                        # char -> fetch host-precomputed tables via one-hot
                        # matmuls. No activations or cell update at all.
                        sa = slabs.tile([128, BLK], bf16, tag="slab", name="sa0")
                        sb = slabs.tile([128, BLK], bf16, tag="slab", name="sb0")
                        if do_dma:
                            nc.sync.dma_start(out=sa[:], in_=oh0_d[grp["a"]])
                            nc.sync.dma_start(out=sb[:], in_=oh0_d[grp["b"]])
                        if pk not in pairc or pairc[pk] is None:
                            cw = 2 * BLK if PAIRTANH else BLK
                            pairc[pk] = statep.tile([128, cw], bf16,
                                                    tag="c", name="c")
                        st = gstate[g] = {"sa": sa, "sb": sb}
                    else:
                        st = gstate[g]
                    cme = pairc[pk]

                    psw = (2 if SIGSPLIT else 4) * BLK
                    ps = psump.tile([128, psw], f32, tag="ps")
                    if do_mm and first:
                        for q, tab in ((0, slice(0, 64)), (1, slice(64, 128))):
                            cs = slice(BLK * q, BLK * q + BLK)
                            nc.tensor.matmul(ps[0:64, cs], t0t[:, tab], sa[:, :],
                                             start=True, stop=True,
                                             tile_position=(0, 0))
                            nc.tensor.matmul(ps[64:128, cs], t0t[:, tab],
                                             sb[:, :], start=True, stop=True,
                                             tile_position=(0, 64))
                    elif do_mm and SIGSPLIT:
                        sig = sigp.tile([128, 4 * BLK], bf16, tag="sig")
                        ps2 = psump.tile([128, psw], f32, tag="ps")
                        for half, pst in ((0, ps), (1, ps2)):
                            for q in range(2):
                                bank = 2 * half + q
                                qs = slice(64 * bank, 64 * bank + 64)
                                cs = slice(BLK * q, BLK * q + BLK)
                                nc.tensor.matmul(pst[0:64, cs], ga[:, qs],
                                                 st["sa"][:, :],
                                                 start=True, stop=True,
                                                 tile_position=(0, 0))
                                if b_act:
                                    nc.tensor.matmul(pst[64:128, cs],
                                                     gb[:, qs],
                                                     st["sb"][:, :],
                                                     start=True, stop=True,
                                                     tile_position=(0, 64))
                            nc.scalar.activation(
                                out=sig[sl, 2 * BLK * half:2 * BLK * (half + 1)],
                                in_=pst[sl, :], func=Sigmoid)
                    elif do_mm:
                        for q in range(4):
                            qs = slice(64 * q, 64 * q + 64)
                            cs = slice(BLK * q, BLK * q + BLK)
                            nc.tensor.matmul(ps[0:64, cs], ga[:, qs],
                                             st["sa"][:, :],
                                             start=True, stop=True,
                                             tile_position=(0, 0))
                            if b_act:
                                nc.tensor.matmul(ps[64:128, cs], gb[:, qs],
                                                 st["sb"][:, :],
                                                 start=True, stop=True,
                                                 tile_position=(0, 64))

                    # next-step rhs tiles (also the h target / output source)
                    na = slabs.tile([128, BLK], bf16, tag="slab", name="na")
                    if do_dma and t + 1 < La:
                        nc.sync.dma_start(out=na[64:128, :],
                                          in_=xsl_d[grp["a"] * MAXL + t + 1])
                    nb_ = None
                    if b_act:
                        nb_ = slabs.tile([128, BLK], bf16, tag="slab", name="nb")
                        if do_dma and t + 1 < Lb:
                            nc.sync.dma_start(out=nb_[0:64, :],
                                              in_=xsl_d[grp["b"] * MAXL + t + 1])

                    if do_act and first:
                        cpy = nc.scalar.copy if T0ACT else None
                        if T0ACT:
                            cpy(cme[sl, coff:coff + BLK], ps[sl, 0:BLK])
                            cpy(na[0:64, :], ps[0:64, BLK:2 * BLK])
                            if b_act:
                                cpy(nb_[64:128, :], ps[64:128, BLK:2 * BLK])
                        else:
                            nc.vector.tensor_copy(cme[sl, coff:coff + BLK],
                                                  ps[sl, 0:BLK])
                            nc.vector.tensor_copy(na[0:64, :],
                                                  ps[0:64, BLK:2 * BLK])
                            if b_act:
                                nc.vector.tensor_copy(nb_[64:128, :],
                                                      ps[64:128, BLK:2 * BLK])
                        if do_dma:
                            if La == 1:
                                nc.sync.dma_start(out=out_d[grp["a"]],
                                                  in_=na[0:64, :])
                            if b_act and Lb == 1:
                                nc.sync.dma_start(out=out_d[grp["b"]],
                                                  in_=nb_[64:128, :])
                            if a["is_ov"] and 0 in a.get("cap_steps", ()):
                                nc.sync.dma_start(out=ov_d[a["ov_idx"] * MAXL],
                                                  in_=na[0:64, :])
                            if b_act and b["is_ov"] and \
                                    0 in b.get("cap_steps", ()):
                                nc.sync.dma_start(out=ov_d[b["ov_idx"] * MAXL],
                                                  in_=nb_[64:128, :])
                    elif do_act:
                        if SIGSPLIT:
                            gs = slice(BLK, 2 * BLK)
                            fs = slice(2 * BLK, 3 * BLK)
                            osl = slice(3 * BLK, 4 * BLK)
                        else:
                            sig = sigp.tile([128, 4 * BLK], bf16, tag="sig")
                            nc.scalar.activation(out=sig[sl, :], in_=ps[sl, :],
                                                 func=Sigmoid)
                            gs = slice(3 * BLK, 4 * BLK)
                            fs = slice(BLK, 2 * BLK)
                            osl = slice(2 * BLK, 3 * BLK)
                        # t1 = (sg - 0.5) * si  (= i*g / 2)
                        t1 = t1p.tile([128, BLK], bf16, tag="t1")
                        nc.vector.scalar_tensor_tensor(
                            t1[sl, :], sig[sl, gs], -0.5,
                            sig[sl, 0:BLK], ADD, MULT)
                        t2 = t2p.tile([128, BLK], bf16, tag="t2")
                        eng = nc.vector if t2dve else nc.gpsimd
                        eng.tensor_mul(t2[sl, :], sig[sl, fs],
                                       cme[sl, coff:coff + BLK])
                        nc.vector.tensor_add(cme[sl, coff:coff + BLK],
                                             t1[sl, :], t2[sl, :])
                        ctx_osl = osl
                        ctx = {"sig": sig, "na": na, "nb_": nb_, "grp": grp,
                               "a": a, "b": b, "La": La, "Lb": Lb,
                               "b_act": b_act, "t": t, "osl": ctx_osl}
                        if mode == "defer":
                            pend[pk] = ctx
                        else:
                            tw = 2 * BLK if PAIRTANH else BLK
                            tch = tcp.tile([128, tw], bf16, tag="tc")
                            if mode == "shared":
                                octx = pend.pop(pk)
                                nc.scalar.activation(out=tch[:, :],
                                                     in_=cme[:, :],
                                                     func=Tanh, scale=2.0)
                                tail(octx, tch, 0)
                                tail(ctx, tch, BLK)
                            else:
                                nc.scalar.activation(
                                    out=tch[sl, coff:coff + BLK],
                                    in_=cme[sl, coff:coff + BLK],
                                    func=Tanh, scale=2.0)
                                tail(ctx, tch, coff)

                    st["sa"] = na
                    if b_act:
                        st["sb"] = nb_

    nc.compile()
    return nc


# --------------------------------------------------------------------------
# Entry point
# --------------------------------------------------------------------------

def kernel(emb, W_ih, W_hh, b_ih, b_hh, chars, lengths):
    from concourse.bass_utils import run_bass_kernel_spmd

    emb = np.asarray(emb, dtype=np.float32)
    W_ih = np.asarray(W_ih, dtype=np.float32)
    W_hh = np.asarray(W_hh, dtype=np.float32)
    b_ih = np.asarray(b_ih, dtype=np.float32)
    b_hh = np.asarray(b_hh, dtype=np.float32)
    chars = np.asarray(chars)
    lengths_np = np.asarray(lengths)

    n = chars.shape[0]

    # --- weight prep -------------------------------------------------------
    # Banks ordered [i, f, o, g]; g-bank scaled by 2 (tanh via sigmoid).
    scale = np.ones((1, GATE4), dtype=np.float32)
    scale[0, _G_BANK * H:(_G_BANK + 1) * H] = 2.0
    WihT = W_ih[_GATE_PERM].T * scale               # [E, 4H]
    WhhT = W_hh[_GATE_PERM].T * scale               # [H, 4H]
    bias = ((b_ih + b_hh)[_GATE_PERM] * scale[0])[None, :]  # [1, 4H]
    gA = np.zeros((128, GATE4), dtype=BF16)
    gA[0:H] = WhhT.astype(BF16)
    gA[H:H + 1] = bias.astype(BF16)
    gA[H + 1:H + 1 + E] = WihT.astype(BF16)
    gB = np.zeros((128, GATE4), dtype=BF16)
    gB[0:E] = WihT.astype(BF16)
    gB[E:E + 1] = bias.astype(BF16)
    gB[64:128] = WhhT.astype(BF16)

    # t=0 tables: gates_0 = G[ch] (h=0), so c0' and h0 are char lookups.
    G = emb @ W_ih.T + b_ih + b_hh                  # [V, 4H] torch order ifgo
    sig_i = 1.0 / (1.0 + np.exp(-G[:, 0:H]))
    sig_g2 = 1.0 / (1.0 + np.exp(-2.0 * G[:, 2 * H:3 * H]))
    sig_o = 1.0 / (1.0 + np.exp(-G[:, 3 * H:4 * H]))
    C0 = (sig_g2 - 0.5) * sig_i                     # = i*g / 2
    H0 = sig_o * np.tanh(2.0 * C0)
    t0tab = np.zeros((128, 128), dtype=BF16)
    t0tab[0:V, 0:H] = C0.astype(BF16)
    t0tab[0:V, H:2 * H] = H0.astype(BF16)

    # --- word assignment ---------------------------------------------------
    plan = _plan(lengths_np)
    blocks, groups, sched = plan["blocks"], plan["groups"], plan["sched"]

    sig = (tuple((b["L"], b["is_ov"], b.get("cap_steps", ())) for b in blocks),
           tuple(sched))
    key = hash(sig)
    if key not in _PROGRAM_CACHE:
        _PROGRAM_CACHE[key] = _build_program(sig, blocks, groups, sched,
                                             plan["n_ov"])
    nc = _PROGRAM_CACHE[key]

    emb16 = emb.astype(BF16)
    xsls = _build_xslabs(plan, chars, emb16)
    oh0s = _build_oh0(plan, chars)
    in_maps = [{"xsl": xsls[c], "oh0": oh0s[c], "ga": gA, "gb": gB,
                "t0": t0tab} for c in range(NCORES)]

    res = run_bass_kernel_spmd(nc, in_maps, core_ids=list(range(NCORES)))
    kernel._last_nc = nc
    kernel._last_in_maps = in_maps

    # --- gather results ----------------------------------------------------
    outs = np.stack([np.asarray(r["out"], dtype=np.float32)
                     for r in res.results])          # [8, nb, H, BLK]
    ovs = np.stack([np.asarray(r["ov"], dtype=np.float32)
                    for r in res.results])           # [8, n_ov*16, H, BLK]

    result = np.empty((n, H), dtype=np.float32)
    for c in range(NCORES):
        for bi, blk in enumerate(blocks):
            words = plan["assign"][c][bi]
            valid = words >= 0
            if not valid.any():
                continue
            w = words[valid]
            cols = np.nonzero(valid)[0]
            if blk["is_ov"]:
                steps = lengths_np[w].astype(np.int64) - 1
                result[w] = ovs[c, blk["ov_idx"] * MAXL + steps, :, cols]
            else:
                result[w] = outs[c, bi, :, cols]
    return result
